# revision 1
# baseline (speedup 1.0000x reference)
"""CrossAttention kernel for 8 trn2 NeuronCores.

Reference:
  q = x @ Wq          [n, vq, h]
  k = y @ Wk          [n, vk, h]
  v = y @ Wv          [n, vk, c]
  out = softmax(q k^T / sqrt(h)) @ v        [n, vq, c]
with N=4, VQ=VK=4096, C=128, H=64, fp32.

Sharding: 8 cores = 4 batches x 2 query halves. Each core gets
x-shard [2048, 128] and the full y[n] [4096, 128], computes out-shard
[2048, 128].

Per-core dataflow (all matmuls fp32r = full-rate PE):
  - transpose x,y tiles on PE (exact) -> xT [c, vq], yT [c, vk]
  - qT [h, vq] = Wq^T xT ; kT [h, vk] = Wk^T yT ; v [vk, c] = (yT)^T Wv
  - flash loop over vq tiles of 512:
      scoresT [vk_t=128, vq=512] = kT_tile^T qT_tile     (K=h=64)
      attnT = exp(scale * scoresT)  (ScalarE, PSUM->SBUF, f32r out)
      outT [c, 512] += v_tile^T ... accumulate over 32 vk tiles
      sums [1, 512] += ones^T attnT
  - sums -> DRAM bounce -> [128, 16] per-partition layout, reciprocal
  - out [vq, c] = transpose(outT) * rsum  (PE transpose + DVE scale)
"""

import sys

sys.path.insert(0, "/opt/trn_rl_repo")

from contextlib import ExitStack

import numpy as np

import concourse.bass as bass
import concourse.tile as tile
from concourse import mybir
from concourse.bass_utils import run_bass_kernel_spmd
from concourse.masks import make_identity

F32 = mybir.dt.float32
F32R = mybir.dt.float32r
P = 128

N, VQ, VK, C, H = 4, 4096, 4096, 128, 64
VQ_PER = VQ // 2          # 2048 queries per core
SCALE = float(H) ** -0.5

# main-loop tiling
VQ_T = 512                # vq tile (psum free dim)
N_VQ_T = VQ_PER // VQ_T   # 4
N_VK_T = VK // P          # 32 vk tiles of 128
CHUNK = 2                 # vk tiles per exp chunk (row-packed pair)


def _split_multi_waits(nc):
    """walrus in this env supports one sync-wait per instruction; hoist
    extras onto same-engine NoOps inserted just before."""
    for fn in nc.m.functions:
        for bb in fn.blocks:
            out = []
            for inst in bb.instructions:
                si = inst.sync_info
                waits = list(si.on_wait) if si and si.on_wait else []
                if len(waits) > 1:
                    for w in waits[:-1]:
                        out.append(mybir.InstNoOp(
                            name=nc.get_next_instruction_name(),
                            engine=inst.engine,
                            ins=[], outs=[],
                            sync_info=mybir.SyncInfo(on_wait=[w], on_update=[]),
                        ))
                    inst.sync_info = mybir.SyncInfo(
                        on_wait=[waits[-1]],
                        on_update=list(si.on_update) if si.on_update else [],
                    )
                out.append(inst)
            bb.instructions = out


def _build():
    nc = bass.Bass()
    x_d = nc.declare_dram_parameter("x", [VQ_PER, C], F32, isOutput=False)
    y_d = nc.declare_dram_parameter("y", [VK, C], F32, isOutput=False)
    wq_d = nc.declare_dram_parameter("Wq", [C, H], F32, isOutput=False)
    wk_d = nc.declare_dram_parameter("Wk", [C, H], F32, isOutput=False)
    wv_d = nc.declare_dram_parameter("Wv", [C, C], F32, isOutput=False)
    o_d = nc.declare_dram_parameter("o", [VQ_PER, C], F32, isOutput=True)

    with tile.TileContext(nc) as tc, ExitStack() as ctx:
        const = ctx.enter_context(tc.tile_pool(name="const", bufs=1))
        persist = ctx.enter_context(tc.tile_pool(name="persist", bufs=1))

        # ---- constants ----
        ident = const.tile([P, P], F32)
        make_identity(nc, ident[:])
        w_stage = const.tile([P, 2 * H + C], F32)
        nc.sync.dma_start(w_stage[:, 0:H], wq_d[:])
        nc.sync.dma_start(w_stage[:, H:2 * H], wk_d[:])
        nc.sync.dma_start(w_stage[:, 2 * H:], wv_d[:])
        w_r = const.tile([P, 4 * H + C], F32R)
        nc.vector.tensor_copy(w_r[:, 0:H], w_stage[:, 0:H])
        nc.vector.tensor_copy(w_r[:, H:2 * H], w_stage[:, 0:H])
        nc.vector.tensor_copy(w_r[:, 2 * H:3 * H], w_stage[:, H:2 * H])
        nc.vector.tensor_copy(w_r[:, 3 * H:4 * H], w_stage[:, H:2 * H])
        nc.vector.tensor_copy(w_r[:, 4 * H:], w_stage[:, 2 * H:])
        wqq_r = w_r[:, 0:2 * H]      # [Wq | Wq] -> duplicated qT rows
        wkk_r = w_r[:, 2 * H:4 * H]  # [Wk | Wk] -> duplicated kT rows
        wv_r = w_r[:, 4 * H:]
        ones_f = const.tile([P, 1], F32)
        nc.vector.memset(ones_f[:], 1.0)
        ones_r = const.tile([P, 1], F32R)
        nc.vector.tensor_copy(ones_r[:], ones_f[:])

        # ---- persistent tensors ----
        qT = persist.tile([P, VQ_PER], F32R)          # [128, 2048] rows 64:128 dup
        kT = persist.tile([P, VK], F32R)              # [128, 4096] rows 64:128 dup
        v_sb = persist.tile([P, N_VK_T * C], F32R)    # [128, 32*128] vk-major
        attnT = persist.tile([P, N_VK_T * VQ_T], F32R)  # [128, 32*512] per vq tile
        outT = persist.tile([P, VQ_PER], F32)         # [c, 2048]
        out_sb = persist.tile([P, (VQ_PER // P) * C], F32)  # [128, 16*128]
        srow = persist.tile([1, VQ_PER], F32)         # softmax sums, vq-flat
        srow2 = persist.tile([1, VQ_PER], F32)        # upper-half partial sums

        # ---- phase 1: load + transpose + project ----
        with ExitStack() as pctx:
            ld = pctx.enter_context(tc.tile_pool(name="ld", bufs=3))
            tp_ps = pctx.enter_context(
                tc.tile_pool(name="tp_ps", bufs=2, space="PSUM"))
            pj_ps = pctx.enter_context(
                tc.tile_pool(name="pj_ps", bufs=2, space="PSUM"))
            v_ps = pctx.enter_context(
                tc.tile_pool(name="v_ps", bufs=2, space="PSUM"))
            xyT = pctx.enter_context(tc.tile_pool(name="xyT", bufs=3))

            def load_transpose(src_ap, n_chunks, proj):
                for ch in range(n_chunks):
                    raw = ld.tile([P, 4, P], F32, tag="raw")
                    nc.sync.dma_start(
                        raw[:],
                        src_ap[ch * 512:(ch + 1) * 512, :]
                        .rearrange("(t p) c -> p t c", p=P),
                    )
                    t_ps = tp_ps.tile([P, 512], F32, tag="tp")
                    for b in range(4):
                        nc.tensor.transpose(
                            t_ps[:, b * P:(b + 1) * P], raw[:, b, :], ident[:])
                    t_sb = xyT.tile([P, 512], F32R, tag="t_sb")
                    nc.vector.tensor_copy(t_sb[:], t_ps[:])
                    proj(ch, t_sb)

            def proj_x(ch, xT_sb):
                q_ps = pj_ps.tile([P, 512], F32, tag="qk")
                nc.tensor.matmul(q_ps[:], wqq_r[:], xT_sb[:], start=True, stop=True)
                nc.vector.tensor_copy(qT[:, ch * 512:(ch + 1) * 512], q_ps[:])

            def proj_y(ch, yT_sb):
                k_ps = pj_ps.tile([P, 512], F32, tag="qk")
                nc.tensor.matmul(k_ps[:], wkk_r[:], yT_sb[:], start=True, stop=True)
                nc.vector.tensor_copy(kT[:, ch * 512:(ch + 1) * 512], k_ps[:])
                vp = v_ps.tile([P, 512], F32, tag="vp")
                for b in range(4):
                    nc.tensor.matmul(
                        vp[:, b * P:(b + 1) * P],
                        yT_sb[:, b * P:(b + 1) * P], wv_r[:],
                        start=True, stop=True)
                nc.vector.tensor_copy(
                    v_sb[:, ch * 512:(ch + 1) * 512], vp[:])

            load_transpose(x_d, VQ_PER // 512, proj_x)
            load_transpose(y_d, VK // 512, proj_y)

        # ---- phase 2: flash loop over vq tiles ----
        # software-pipelined emission: PE stream = sc(n+1) before pv/sm(n),
        # so the PE never blocks on the activation of the current chunk.
        with ExitStack() as mctx:
            sc_ps = mctx.enter_context(
                tc.tile_pool(name="sc_ps", bufs=2, space="PSUM"))
            pv_ps = mctx.enter_context(
                tc.tile_pool(name="pv_ps", bufs=2, space="PSUM"))
            sm_ps = mctx.enter_context(
                tc.tile_pool(name="sm_ps", bufs=1, space="PSUM"))

            sm2_ps = mctx.enter_context(
                tc.tile_pool(name="sm2_ps", bufs=1, space="PSUM"))
            pv_tiles = [None] * N_VQ_T
            starts = list(range(0, N_VK_T, CHUNK))
            work = [(j, s) for j in range(N_VQ_T) for s in starts]

            def emit_scores_exp(j, s):
                sc = sc_ps.tile([P, CHUNK * VQ_T], F32, tag="sc")
                nc.tensor.matmul(
                    sc[:, 0:VQ_T],
                    kT[0:64, s * P:(s + 1) * P],
                    qT[0:64, j * VQ_T:(j + 1) * VQ_T],
                    start=True, stop=True)
                nc.tensor.matmul(
                    sc[:, VQ_T:2 * VQ_T],
                    kT[64:128, (s + 1) * P:(s + 2) * P],
                    qT[64:128, j * VQ_T:(j + 1) * VQ_T],
                    start=True, stop=True, tile_position=(64, 0))
                nc.scalar.activation(
                    attnT[:, s * VQ_T:(s + 2) * VQ_T],
                    sc[:],
                    mybir.ActivationFunctionType.Exp, scale=SCALE)

            def emit_pv_sm(j, s):
                if s == 0:
                    pv = pv_ps.tile([P, VQ_T], F32, tag="pv", name=f"pv{j}")
                    sm = sm_ps.tile([1, VQ_T], F32, tag="sm", name=f"sm{j}")
                    sm2 = sm2_ps.tile([1, VQ_T], F32, tag="sm2", name=f"sm2_{j}")
                    pv_tiles[j] = (pv, sm, sm2)
                pv, sm, sm2 = pv_tiles[j]
                for ii in range(CHUNK):
                    i = s + ii
                    a_sl = attnT[:, i * VQ_T:(i + 1) * VQ_T]
                    nc.tensor.matmul(
                        pv[:], v_sb[:, i * C:(i + 1) * C], a_sl,
                        start=(i == 0), stop=(i == N_VK_T - 1))
                for ii in range(CHUNK):
                    i = s + ii
                    a_sl = attnT[:, i * VQ_T:(i + 1) * VQ_T]
                    # concurrent pair: lower half rows -> sm, upper -> sm2
                    nc.tensor.matmul(
                        sm[:], ones_r[0:64, :], a_sl[0:64, :],
                        start=(i == 0), stop=(i == N_VK_T - 1))
                    nc.tensor.matmul(
                        sm2[:], ones_r[64:128, :], a_sl[64:128, :],
                        start=(i == 0), stop=(i == N_VK_T - 1),
                        tile_position=(64, 0))
                if s == starts[-1]:
                    nc.vector.tensor_copy(
                        outT[:, j * VQ_T:(j + 1) * VQ_T], pv[:])
                    nc.vector.tensor_copy(
                        srow[:, j * VQ_T:(j + 1) * VQ_T], sm[:])
                    nc.vector.tensor_copy(
                        srow2[:, j * VQ_T:(j + 1) * VQ_T], sm2[:])

            for n, (j, s) in enumerate(work):
                emit_scores_exp(j, s)
                if n > 0:
                    emit_pv_sm(*work[n - 1])
            emit_pv_sm(*work[-1])

        # ---- phase 3: sums transpose + reciprocal + out transpose ----
        with ExitStack() as fctx:
            fin = fctx.enter_context(tc.tile_pool(name="fin", bufs=1))
            f_ps = fctx.enter_context(
                tc.tile_pool(name="f_ps", bufs=4, space="PSUM"))
            s_ps = fctx.enter_context(
                tc.tile_pool(name="s_ps", bufs=1, space="PSUM"))

            n_ot = VQ_PER // P  # 16 output blocks of [128 vq, 128 c]
            # fold the two half-row partial sums, then transpose
            nc.vector.tensor_tensor(
                out=srow[:], in0=srow[:], in1=srow2[:],
                op=mybir.AluOpType.add)
            # transpose sums row [1, 2048] -> [128, 16] via PE row-transposes
            sT_ps = s_ps.tile([P, n_ot], F32)
            for t in range(n_ot):
                nc.tensor.transpose(
                    sT_ps[:, t:t + 1], srow[0:1, t * P:(t + 1) * P],
                    ones_f[0:1, 0:1])
            sumsT = fin.tile([P, n_ot], F32)
            nc.vector.tensor_copy(sumsT[:], sT_ps[:])
            rsum = fin.tile([P, n_ot], F32)
            nc.vector.reciprocal(rsum[:], sumsT[:])

            for t in range(n_ot):
                o_ps = f_ps.tile([P, P], F32)
                nc.tensor.transpose(
                    o_ps[:], outT[:, t * P:(t + 1) * P], ident[:])
                nc.vector.tensor_scalar(
                    out=out_sb[:, t * P:(t + 1) * P], in0=o_ps[:],
                    scalar1=rsum[:, t:t + 1], scalar2=None,
                    op0=mybir.AluOpType.mult)
            nc.sync.dma_start(
                o_d[:].rearrange("(t p) c -> p t c", p=P),
                out_sb[:].rearrange("p (t c) -> p t c", c=C),
            )

    _split_multi_waits(nc)
    return nc


_NC = None


def _get_nc():
    global _NC
    if _NC is None:
        _NC = _build()
    return _NC


def kernel(x, y, Wq, Wk, Wv):
    x = np.ascontiguousarray(x, dtype=np.float32)
    y = np.ascontiguousarray(y, dtype=np.float32)
    Wq = np.ascontiguousarray(Wq, dtype=np.float32)
    Wk = np.ascontiguousarray(Wk, dtype=np.float32)
    Wv = np.ascontiguousarray(Wv, dtype=np.float32)

    nc = _get_nc()
    core_ids = list(range(8))
    in_maps = []
    for core in core_ids:
        n, half = core // 2, core % 2
        in_maps.append({
            "x": x[n, half * VQ_PER:(half + 1) * VQ_PER, :],
            "y": y[n],
            "Wq": Wq, "Wk": Wk, "Wv": Wv,
        })
    res = run_bass_kernel_spmd(nc, in_maps, core_ids)
    out = np.empty((N, VQ, C), dtype=np.float32)
    for core in core_ids:
        n, half = core // 2, core % 2
        out[n, half * VQ_PER:(half + 1) * VQ_PER, :] = res.results[core]["o"]
    return out



# revision 4
# speedup vs baseline: 1.1331x; 1.1331x over previous
"""CrossAttention kernel for 8 trn2 NeuronCores (v2).

Reference:
  q = x @ Wq          [n, vq, h]
  k = y @ Wk          [n, vk, h]
  v = y @ Wv          [n, vk, c]
  out = softmax(q k^T / sqrt(h)) @ v        [n, vq, c]
with N=4, VQ=VK=4096, C=128, H=64, fp32.

Sharding: 8 cores = 4 batches x 2 query halves.

v2 structure (vs v1 baseline at 154us):
  - host pre-transposes x,y (feeds xT [c,vq], yT [c,vk], y [vk,c]); no PE
    transposes / PSUM bounces for input staging.
  - v-projection folded into the flash loop: z = sum_i y_i^T attn_i
    accumulates [c, vq] per vq tile; out^T = Wv^T z at the end. The v=y@Wv
    projection and its PSUM copies disappear.
  - softmax sums via single K=128 ones-matmuls (1 PSUM bank), denominators
    and final transpose applied on HOST: device returns unnormalized
    oT [c, vq] and sums [1, vq].
  - PSUM: scores pool 3 bufs x 2 banks + z 1 + sums 1 = 8 banks; 3-deep
    score lookahead keeps ScalarE (the bottleneck: 64 exps of ~1.1us)
    gapless.
  - input loads/projections interleaved into the first vq tile's flash
    chunks so DMA/PE staging hides behind ScalarE work.
"""

import sys

sys.path.insert(0, "/opt/trn_rl_repo")

from contextlib import ExitStack

import numpy as np

import concourse.bass as bass
import concourse.tile as tile
from concourse import mybir
from concourse.bass_utils import run_bass_kernel_spmd

F32 = mybir.dt.float32
F32R = mybir.dt.float32r
P = 128

N, VQ, VK, C, H = 4, 4096, 4096, 128, 64
VQ_PER = VQ // 2          # 2048 queries per core
SCALE = float(H) ** -0.5

VQ_T = 512                # vq tile (psum free dim)
N_VQ_T = VQ_PER // VQ_T   # 4
N_VK_T = VK // P          # 32 vk tiles of 128
CHUNK = 2                 # vk tiles per exp chunk
N_CH = N_VK_T // CHUNK    # 16 chunks per vq tile


def _split_multi_waits(nc):
    """walrus in this env supports one sync-wait per instruction; hoist
    extras onto same-engine NoOps inserted just before."""
    for fn in nc.m.functions:
        for bb in fn.blocks:
            out = []
            for inst in bb.instructions:
                si = inst.sync_info
                waits = list(si.on_wait) if si and si.on_wait else []
                if len(waits) > 1:
                    for w in waits[:-1]:
                        out.append(mybir.InstNoOp(
                            name=nc.get_next_instruction_name(),
                            engine=inst.engine,
                            ins=[], outs=[],
                            sync_info=mybir.SyncInfo(on_wait=[w], on_update=[]),
                        ))
                    inst.sync_info = mybir.SyncInfo(
                        on_wait=[waits[-1]],
                        on_update=list(si.on_update) if si.on_update else [],
                    )
                out.append(inst)
            bb.instructions = out


def _build():
    nc = bass.Bass()
    xT_d = nc.declare_dram_parameter("xT", [C, VQ_PER], F32R, isOutput=False)
    yT_d = nc.declare_dram_parameter("yT", [C, VK], F32R, isOutput=False)
    y_d = nc.declare_dram_parameter("y", [VK, C], F32R, isOutput=False)
    wqq_d = nc.declare_dram_parameter("wqq", [C, 2 * H], F32R, isOutput=False)
    wkk_d = nc.declare_dram_parameter("wkk", [C, 2 * H], F32R, isOutput=False)
    wv_d = nc.declare_dram_parameter("wv", [C, C], F32R, isOutput=False)
    oT_d = nc.declare_dram_parameter("oT", [C, VQ_PER], F32, isOutput=True)
    sums_d = nc.declare_dram_parameter("sums", [1, VQ_PER], F32, isOutput=True)

    with tile.TileContext(nc) as tc, ExitStack() as ctx:
        const = ctx.enter_context(tc.tile_pool(name="const", bufs=1))
        persist = ctx.enter_context(tc.tile_pool(name="persist", bufs=1))

        # ---- constants ----
        w_sb = const.tile([P, 3 * P], F32R)
        nc.sync.dma_start(w_sb[:, 0:P], wqq_d[:])
        nc.sync.dma_start(w_sb[:, P:2 * P], wkk_d[:])
        nc.sync.dma_start(w_sb[:, 2 * P:], wv_d[:])
        wqq_sb = w_sb[:, 0:P]
        wkk_sb = w_sb[:, P:2 * P]
        wv_sb = w_sb[:, 2 * P:]
        ones_f = const.tile([P, 1], F32)
        nc.vector.memset(ones_f[:], 1.0)
        ones_r = const.tile([P, 1], F32R)
        nc.vector.tensor_copy(ones_r[:], ones_f[:])

        # ---- persistent tensors ----
        qT = persist.tile([P, VQ_PER], F32R)          # rows 64:128 dup
        kT = persist.tile([P, VK], F32R)              # rows 64:128 dup
        y_sb = persist.tile([P, N_VK_T, P], F32R)     # y tiles [vk, c]
        attn = persist.tile([P, N_VK_T * VQ_T], F32R)  # per vq tile, rotating
        z_sb = persist.tile([P, VQ_PER], F32R)        # z = y^T attn  [c, vq]
        oT_sb = persist.tile([P, VQ_PER], F32)        # Wv^T z        [c, vq]
        srow = persist.tile([1, VQ_PER], F32)         # softmax sums

        with ExitStack() as mctx:
            # sc pool is also used (same shape/tag) for projection bounces
            sc_ps = mctx.enter_context(
                tc.tile_pool(name="sc_ps", bufs=3, space="PSUM"))
            z_ps = mctx.enter_context(
                tc.tile_pool(name="z_ps", bufs=1, space="PSUM"))
            sm_ps = mctx.enter_context(
                tc.tile_pool(name="sm_ps", bufs=1, space="PSUM"))

            # ---- staging: DMA + projections (emitted interleaved below) --
            def dma_x(chp):  # chp = chunk pair index (0..1), 1024 cols
                sl = slice(chp * 1024, (chp + 1) * 1024)
                nc.sync.dma_start(qx_stage[chp][:], xT_d[:, sl])

            def dma_y(chp):  # chp 0..3, 1024 vk cols of yT + 1024 raw rows
                sl = slice(chp * 1024, (chp + 1) * 1024)
                nc.sync.dma_start(ky_stage[chp][:], yT_d[:, sl])
                nc.sync.dma_start(
                    y_sb[:, 8 * chp:8 * (chp + 1), :],
                    y_d[sl, :].rearrange("(t p) c -> p t c", p=P),
                )

            def proj_x(chp):
                ps = sc_ps.tile([P, 1024], F32, tag="sc", name=f"pjx{chp}")
                for b in range(2):
                    nc.tensor.matmul(
                        ps[:, b * 512:(b + 1) * 512], wqq_sb,
                        qx_stage[chp][:, b * 512:(b + 1) * 512],
                        start=True, stop=True)
                sl = slice(chp * 1024, (chp + 1) * 1024)
                nc.vector.tensor_copy(qT[:, sl], ps[:])

            def proj_y(chp):
                ps = sc_ps.tile([P, 1024], F32, tag="sc", name=f"pjy{chp}")
                for b in range(2):
                    nc.tensor.matmul(
                        ps[:, b * 512:(b + 1) * 512], wkk_sb,
                        ky_stage[chp][:, b * 512:(b + 1) * 512],
                        start=True, stop=True)
                sl = slice(chp * 1024, (chp + 1) * 1024)
                nc.vector.tensor_copy(kT[:, sl], ps[:])

            # staging SBUF for xT/yT chunk pairs (raw f32r straight from DMA)
            stage = mctx.enter_context(tc.tile_pool(name="stage", bufs=1))
            qx_stage = [stage.tile([P, 1024], F32R, name=f"qx{i}")
                        for i in range(2)]
            ky_stage = [stage.tile([P, 1024], F32R, name=f"ky{i}")
                        for i in range(4)]

            # ---- flash loop ----
            z_tiles = [None] * N_VQ_T

            def emit_scores_exp(j, c):
                sc = sc_ps.tile([P, CHUNK * VQ_T], F32, tag="sc")
                s = CHUNK * c
                nc.tensor.matmul(
                    sc[:, 0:VQ_T],
                    kT[0:64, s * P:(s + 1) * P],
                    qT[0:64, j * VQ_T:(j + 1) * VQ_T],
                    start=True, stop=True)
                nc.tensor.matmul(
                    sc[:, VQ_T:2 * VQ_T],
                    kT[64:128, (s + 1) * P:(s + 2) * P],
                    qT[64:128, j * VQ_T:(j + 1) * VQ_T],
                    start=True, stop=True, tile_position=(64, 0))
                nc.scalar.activation(
                    attn[:, s * VQ_T:(s + 2) * VQ_T],
                    sc[:],
                    mybir.ActivationFunctionType.Exp, scale=SCALE)

            def emit_consume(j, c):
                if c == 0:
                    zp = z_ps.tile([P, VQ_T], F32, tag="z", name=f"z{j}")
                    sm = sm_ps.tile([1, VQ_T], F32, tag="sm", name=f"sm{j}")
                    z_tiles[j] = (zp, sm)
                zp, sm = z_tiles[j]
                for ii in range(CHUNK):
                    i = CHUNK * c + ii
                    a_sl = attn[:, i * VQ_T:(i + 1) * VQ_T]
                    nc.tensor.matmul(
                        zp[:], y_sb[:, i, :], a_sl,
                        start=(i == 0), stop=(i == N_VK_T - 1))
                    nc.tensor.matmul(
                        sm[:], ones_r[:], a_sl,
                        start=(i == 0), stop=(i == N_VK_T - 1))
                if c == N_CH - 1:
                    nc.vector.tensor_copy(
                        z_sb[:, j * VQ_T:(j + 1) * VQ_T], zp[:])
                    nc.vector.tensor_copy(
                        srow[:, j * VQ_T:(j + 1) * VQ_T], sm[:])

            # background staging tasks spread over early flash chunks.
            # proj_y(chp) provides kT for flash chunks 4*chp..4*chp+3 of j=0;
            # emitted at flash index <= 4*chp - 1 below.
            pre = [lambda: dma_x(0), lambda: dma_y(0), lambda: proj_x(0),
                   lambda: proj_y(0), lambda: dma_y(1)]
            bg = [
                lambda: proj_y(1),          # before flash 4 (kT tiles 8..15)
                lambda: dma_y(2),
                lambda: proj_y(2),          # before flash 8
                lambda: dma_x(1),
                lambda: dma_y(3),
                lambda: proj_y(3),          # before flash 12
                lambda: proj_x(1),
            ]
            for t in pre:
                t()

            work = [(j, c) for j in range(N_VQ_T) for c in range(N_CH)]
            for n, (j, c) in enumerate(work):
                emit_scores_exp(j, c)
                if n < len(bg):
                    bg[n]()
                if n > 0:
                    emit_consume(*work[n - 1])
            emit_consume(*work[-1])

        # ---- tail: oT = Wv^T z, store ----
        with ExitStack() as fctx:
            f_ps = fctx.enter_context(
                tc.tile_pool(name="f_ps", bufs=2, space="PSUM"))
            for j in range(N_VQ_T):
                sl = slice(j * VQ_T, (j + 1) * VQ_T)
                o2 = f_ps.tile([P, VQ_T], F32, tag="o2")
                nc.tensor.matmul(o2[:], wv_sb, z_sb[:, sl],
                                 start=True, stop=True)
                if j % 2:
                    nc.scalar.copy(oT_sb[:, sl], o2[:])
                else:
                    nc.vector.tensor_copy(oT_sb[:, sl], o2[:])
                nc.sync.dma_start(oT_d[:, sl], oT_sb[:, sl])
            nc.sync.dma_start(sums_d[:], srow[:])

    _split_multi_waits(nc)
    return nc


_NC = None


def _get_nc():
    global _NC
    if _NC is None:
        _NC = _build()
    return _NC


def make_in_maps(x, y, Wq, Wk, Wv):
    x = np.ascontiguousarray(x, dtype=np.float32)
    y = np.ascontiguousarray(y, dtype=np.float32)
    wqq = np.ascontiguousarray(
        np.concatenate([Wq, Wq], axis=1), dtype=np.float32)
    wkk = np.ascontiguousarray(
        np.concatenate([Wk, Wk], axis=1), dtype=np.float32)
    wv = np.ascontiguousarray(Wv, dtype=np.float32)
    in_maps = []
    for core in range(8):
        n, half = core // 2, core % 2
        in_maps.append({
            "xT": np.ascontiguousarray(
                x[n, half * VQ_PER:(half + 1) * VQ_PER, :].T),
            "yT": np.ascontiguousarray(y[n].T),
            "y": y[n],
            "wqq": wqq, "wkk": wkk, "wv": wv,
        })
    return in_maps


def finish(results):
    """Host-side epilogue: normalize + transpose per core shard."""
    out = np.empty((N, VQ, C), dtype=np.float32)
    for core in range(8):
        n, half = core // 2, core % 2
        r = results[core]
        out[n, half * VQ_PER:(half + 1) * VQ_PER, :] = (
            r["oT"] / r["sums"]).T
    return out


def kernel(x, y, Wq, Wk, Wv):
    nc = _get_nc()
    in_maps = make_in_maps(x, y, Wq, Wk, Wv)
    res = run_bass_kernel_spmd(nc, in_maps, list(range(8)))
    return finish(res.results)


# revision 5
# speedup vs baseline: 1.4813x; 1.3073x over previous
"""CrossAttention kernel for 8 trn2 NeuronCores (v3).

Reference:
  q = x @ Wq          [n, vq, h]
  k = y @ Wk          [n, vk, h]
  v = y @ Wv          [n, vk, c]
  out = softmax(q k^T / sqrt(h)) @ v        [n, vq, c]
with N=4, VQ=VK=4096, C=128, H=64, fp32.

Sharding: 8 cores = 4 batches x 2 query halves.

v3 structure (v1 154us, v2 135us):
  - PE streams serialize (~N cycles each, no tile_position concurrency),
    so the per-chunk PE cost is exactly the streamed columns. Minimum
    streams: scores (2x512) + z (2x512) per chunk. Softmax sums moved OFF
    the PE: the exp writes attn in bf16 and the idle VectorE accumulates
    per-chunk partial sums (bf16 2x mode); one tiny PE ones-matmul per vq
    tile does the final partition reduction.
  - host feeds xT/yT (f32r) and y tiles (bf16); z = sum_i y_i^T attn_i in
    bf16, out^T = Wv^T z at the end; host normalizes + transposes.
  - PSUM: scores 3 bufs x 2 banks + z 1 + sums 1 = 8 banks.
"""

import sys

sys.path.insert(0, "/opt/trn_rl_repo")

from contextlib import ExitStack

import ml_dtypes
import numpy as np

import concourse.bass as bass
import concourse.tile as tile
from concourse import mybir
from concourse.bass_utils import run_bass_kernel_spmd

F32 = mybir.dt.float32
F32R = mybir.dt.float32r
BF16 = mybir.dt.bfloat16
P = 128

N, VQ, VK, C, H = 4, 4096, 4096, 128, 64
VQ_PER = VQ // 2          # 2048 queries per core
SCALE = float(H) ** -0.5

VQ_T = 512                # vq tile (psum free dim)
N_VQ_T = VQ_PER // VQ_T   # 4
N_VK_T = VK // P          # 32 vk tiles of 128
CHUNK = 2                 # vk tiles per exp chunk
N_CH = N_VK_T // CHUNK    # 16 chunks per vq tile


def _split_multi_waits(nc):
    """walrus in this env supports one sync-wait per instruction; hoist
    extras onto same-engine NoOps inserted just before."""
    for fn in nc.m.functions:
        for bb in fn.blocks:
            out = []
            for inst in bb.instructions:
                si = inst.sync_info
                waits = list(si.on_wait) if si and si.on_wait else []
                if len(waits) > 1:
                    for w in waits[:-1]:
                        out.append(mybir.InstNoOp(
                            name=nc.get_next_instruction_name(),
                            engine=inst.engine,
                            ins=[], outs=[],
                            sync_info=mybir.SyncInfo(on_wait=[w], on_update=[]),
                        ))
                    inst.sync_info = mybir.SyncInfo(
                        on_wait=[waits[-1]],
                        on_update=list(si.on_update) if si.on_update else [],
                    )
                out.append(inst)
            bb.instructions = out


def _build():
    nc = bass.Bass()
    xT_d = nc.declare_dram_parameter("xT", [C, VQ_PER], F32R, isOutput=False)
    yT_d = nc.declare_dram_parameter("yT", [C, VK], F32R, isOutput=False)
    y_d = nc.declare_dram_parameter("y", [VK, C], BF16, isOutput=False)
    wqq_d = nc.declare_dram_parameter("wqq", [C, 2 * H], F32R, isOutput=False)
    wkk_d = nc.declare_dram_parameter("wkk", [C, 2 * H], F32R, isOutput=False)
    wv_d = nc.declare_dram_parameter("wv", [C, C], F32R, isOutput=False)
    oT_d = nc.declare_dram_parameter("oT", [C, VQ_PER], F32, isOutput=True)
    sums_d = nc.declare_dram_parameter("sums", [1, VQ_PER], F32, isOutput=True)

    with tile.TileContext(nc) as tc, ExitStack() as ctx:
        const = ctx.enter_context(tc.tile_pool(name="const", bufs=1))
        persist = ctx.enter_context(tc.tile_pool(name="persist", bufs=1))

        # ---- constants ----
        w_sb = const.tile([P, 3 * P], F32R)
        nc.sync.dma_start(w_sb[:, 0:P], wqq_d[:])
        nc.sync.dma_start(w_sb[:, P:2 * P], wkk_d[:])
        nc.sync.dma_start(w_sb[:, 2 * P:], wv_d[:])
        wqq_sb = w_sb[:, 0:P]
        wkk_sb = w_sb[:, P:2 * P]
        wv_sb = w_sb[:, 2 * P:]
        ones_f = const.tile([P, 1], F32)
        nc.vector.memset(ones_f[:], 1.0)
        ones_b = const.tile([P, 1], BF16)
        nc.vector.tensor_copy(ones_b[:], ones_f[:])

        # ---- persistent tensors ----
        qT = persist.tile([P, VQ_PER], F32R)          # rows 64:128 dup
        kT = persist.tile([P, VK], F32R)              # rows 64:128 dup
        y_sb = persist.tile([P, N_VK_T, P], BF16)     # y tiles [vk, c]
        attn = persist.tile([P, N_VK_T * VQ_T], BF16)  # per vq tile, rotating
        acc = persist.tile([P, CHUNK * VQ_T], BF16)   # DVE partial sums
        acc2 = persist.tile([P, VQ_T], BF16)          # folded partial sums
        z_sb = persist.tile([P, VQ_PER], F32R)        # z = y^T attn  [c, vq]
        oT_sb = persist.tile([P, VQ_PER], F32)        # Wv^T z        [c, vq]
        srow = persist.tile([1, VQ_PER], F32)         # softmax sums

        with ExitStack() as mctx:
            # sc pool is also used (same shape/tag) for projection bounces
            sc_ps = mctx.enter_context(
                tc.tile_pool(name="sc_ps", bufs=3, space="PSUM"))
            z_ps = mctx.enter_context(
                tc.tile_pool(name="z_ps", bufs=1, space="PSUM"))
            sm_ps = mctx.enter_context(
                tc.tile_pool(name="sm_ps", bufs=1, space="PSUM"))

            # staging SBUF for xT/yT chunk pairs (f32r straight from DMA)
            stage = mctx.enter_context(tc.tile_pool(name="stage", bufs=1))
            qx_stage = [stage.tile([P, 1024], F32R, name=f"qx{i}")
                        for i in range(2)]
            ky_stage = [stage.tile([P, 1024], F32R, name=f"ky{i}")
                        for i in range(4)]

            # ---- staging: DMA + projections (emitted interleaved below) --
            def dma_x(chp):  # chp = chunk pair index (0..1), 1024 cols
                sl = slice(chp * 1024, (chp + 1) * 1024)
                nc.sync.dma_start(qx_stage[chp][:], xT_d[:, sl])

            def dma_yT(chp):  # chp 0..3, 1024 vk cols of yT
                sl = slice(chp * 1024, (chp + 1) * 1024)
                nc.sync.dma_start(ky_stage[chp][:], yT_d[:, sl])

            def dma_y_raw(chp):  # chp 0..3, 1024 raw rows (8 vk tiles), bf16
                sl = slice(chp * 1024, (chp + 1) * 1024)
                nc.scalar.dma_start(
                    y_sb[:, 8 * chp:8 * (chp + 1), :],
                    y_d[sl, :].rearrange("(t p) c -> p t c", p=P),
                )

            def proj_x(chp):
                ps = sc_ps.tile([P, 1024], F32, tag="sc", name=f"pjx{chp}")
                for b in range(2):
                    nc.tensor.matmul(
                        ps[:, b * 512:(b + 1) * 512], wqq_sb,
                        qx_stage[chp][:, b * 512:(b + 1) * 512],
                        start=True, stop=True)
                sl = slice(chp * 1024, (chp + 1) * 1024)
                nc.vector.tensor_copy(qT[:, sl], ps[:])

            def proj_y(chp):
                ps = sc_ps.tile([P, 1024], F32, tag="sc", name=f"pjy{chp}")
                for b in range(2):
                    nc.tensor.matmul(
                        ps[:, b * 512:(b + 1) * 512], wkk_sb,
                        ky_stage[chp][:, b * 512:(b + 1) * 512],
                        start=True, stop=True)
                sl = slice(chp * 1024, (chp + 1) * 1024)
                nc.vector.tensor_copy(kT[:, sl], ps[:])

            # ---- flash loop ----
            z_tiles = [None] * N_VQ_T

            def emit_scores_exp(j, c):
                sc = sc_ps.tile([P, CHUNK * VQ_T], F32, tag="sc")
                s = CHUNK * c
                nc.tensor.matmul(
                    sc[:, 0:VQ_T],
                    kT[0:64, s * P:(s + 1) * P],
                    qT[0:64, j * VQ_T:(j + 1) * VQ_T],
                    start=True, stop=True)
                nc.tensor.matmul(
                    sc[:, VQ_T:2 * VQ_T],
                    kT[64:128, (s + 1) * P:(s + 2) * P],
                    qT[64:128, j * VQ_T:(j + 1) * VQ_T],
                    start=True, stop=True, tile_position=(64, 0))
                nc.scalar.activation(
                    attn[:, s * VQ_T:(s + 2) * VQ_T],
                    sc[:],
                    mybir.ActivationFunctionType.Exp, scale=SCALE)

            def emit_consume(j, c):
                if c == 0:
                    zp = z_ps.tile([P, VQ_T], F32, tag="z", name=f"z{j}")
                    z_tiles[j] = zp
                zp = z_tiles[j]
                for ii in range(CHUNK):
                    i = CHUNK * c + ii
                    a_sl = attn[:, i * VQ_T:(i + 1) * VQ_T]
                    nc.tensor.matmul(
                        zp[:], y_sb[:, i, :], a_sl,
                        start=(i == 0), stop=(i == N_VK_T - 1))
                # VectorE partial-sum accumulation (both tiles in one op)
                ch_sl = attn[:, CHUNK * c * VQ_T:CHUNK * (c + 1) * VQ_T]
                if c == 0:
                    nc.vector.tensor_copy(acc[:], ch_sl)
                else:
                    nc.vector.tensor_tensor(
                        out=acc[:], in0=acc[:], in1=ch_sl,
                        op=mybir.AluOpType.add)
                if c == N_CH - 1:
                    # fold halves, partition-reduce on PE, evacuate
                    nc.vector.tensor_tensor(
                        out=acc2[:], in0=acc[:, 0:VQ_T], in1=acc[:, VQ_T:],
                        op=mybir.AluOpType.add)
                    sm = sm_ps.tile([1, VQ_T], F32, tag="sm", name=f"sm{j}")
                    nc.tensor.matmul(sm[:], ones_b[:], acc2[:],
                                     start=True, stop=True)
                    nc.vector.tensor_copy(
                        z_sb[:, j * VQ_T:(j + 1) * VQ_T], zp[:])
                    nc.vector.tensor_copy(
                        srow[:, j * VQ_T:(j + 1) * VQ_T], sm[:])

            # background staging tasks spread over early flash chunks.
            pre = [lambda: dma_x(0), lambda: dma_yT(0),
                   lambda: dma_y_raw(0), lambda: dma_y_raw(1),
                   lambda: dma_y_raw(2), lambda: dma_y_raw(3),
                   lambda: proj_x(0), lambda: proj_y(0)]
            bg = [
                lambda: dma_yT(1),
                lambda: proj_y(1),          # kT for flash chunks 4..7
                lambda: dma_yT(2),
                lambda: dma_x(1),
                lambda: proj_y(2),          # flash 8..11
                lambda: dma_yT(3),
                lambda: proj_x(1),
                lambda: proj_y(3),          # flash 12..15
            ]
            for t in pre:
                t()

            work = [(j, c) for j in range(N_VQ_T) for c in range(N_CH)]
            for n, (j, c) in enumerate(work):
                emit_scores_exp(j, c)
                if n < len(bg):
                    bg[n]()
                if n > 0:
                    emit_consume(*work[n - 1])
            emit_consume(*work[-1])

        # ---- tail: oT = Wv^T z, store ----
        with ExitStack() as fctx:
            f_ps = fctx.enter_context(
                tc.tile_pool(name="f_ps", bufs=2, space="PSUM"))
            for j in range(N_VQ_T):
                sl = slice(j * VQ_T, (j + 1) * VQ_T)
                o2 = f_ps.tile([P, VQ_T], F32, tag="o2")
                nc.tensor.matmul(o2[:], wv_sb, z_sb[:, sl],
                                 start=True, stop=True)
                if j % 2:
                    nc.scalar.copy(oT_sb[:, sl], o2[:])
                else:
                    nc.vector.tensor_copy(oT_sb[:, sl], o2[:])
                nc.sync.dma_start(oT_d[:, sl], oT_sb[:, sl])
            nc.sync.dma_start(sums_d[:], srow[:])

    _split_multi_waits(nc)
    return nc


_NC = None


def _get_nc():
    global _NC
    if _NC is None:
        _NC = _build()
    return _NC


def make_in_maps(x, y, Wq, Wk, Wv):
    x = np.ascontiguousarray(x, dtype=np.float32)
    y = np.ascontiguousarray(y, dtype=np.float32)
    wqq = np.ascontiguousarray(
        np.concatenate([Wq, Wq], axis=1), dtype=np.float32)
    wkk = np.ascontiguousarray(
        np.concatenate([Wk, Wk], axis=1), dtype=np.float32)
    wv = np.ascontiguousarray(Wv, dtype=np.float32)
    in_maps = []
    for core in range(8):
        n, half = core // 2, core % 2
        in_maps.append({
            "xT": np.ascontiguousarray(
                x[n, half * VQ_PER:(half + 1) * VQ_PER, :].T),
            "yT": np.ascontiguousarray(y[n].T),
            "y": y[n].astype(ml_dtypes.bfloat16),
            "wqq": wqq, "wkk": wkk, "wv": wv,
        })
    return in_maps


def finish(results):
    """Host-side epilogue: normalize + transpose per core shard."""
    out = np.empty((N, VQ, C), dtype=np.float32)
    for core in range(8):
        n, half = core // 2, core % 2
        r = results[core]
        out[n, half * VQ_PER:(half + 1) * VQ_PER, :] = (
            r["oT"] / r["sums"]).T
    return out


def kernel(x, y, Wq, Wk, Wv):
    nc = _get_nc()
    in_maps = make_in_maps(x, y, Wq, Wk, Wv)
    res = run_bass_kernel_spmd(nc, in_maps, list(range(8)))
    return finish(res.results)
